# revision 1
# baseline (speedup 1.0000x reference)
"""Causal self-attention (GQA, rope) on 8 Trainium2 NeuronCores.

Sharding: tensor-parallel over the 4 kv-head groups x data-parallel over the
batch of 2.  Core c handles batch b = c // 4 and kv-group g = c % 4:

  - q/k/v projections for the group's 4 q-heads + 1 kv-head,
  - rope, causal flash-style attention (unnormalized softmax: e = exp(s),
    y = (e @ v) * (1 / (e @ 1)) -- safe here because scores are O(1)),
  - partial output projection out_partial = y_g @ wproj[:, cols_g].T.

The host sums the 4 group partials per batch element.

All matmuls run in bf16 with fp32 PSUM accumulation.  Activations are cast
host-side; x / weights are pre-transposed host-side so the contraction dim
lands on SBUF partitions without any on-device fp32 transposes.
"""

import numpy as np
import ml_dtypes

BF16 = ml_dtypes.bfloat16

T = 2048
C = 2048
HD = 128
N_KV = 4
N_REP = 4
O_G = N_REP * HD  # 512 q-dims per group
TC = 512  # t-chunk (psum bank width in fp32)
N_TC = T // TC  # 4
N_KT = C // 128  # 16 contraction tiles
SCALE = float(1.0 / np.sqrt(HD))

_compiled = None


def _build():
    import concourse.bacc as bacc
    import concourse.mybir as mybir
    import concourse.tile as tile
    from concourse.masks import make_identity

    f32 = mybir.dt.float32
    bf16 = mybir.dt.bfloat16

    nc = bacc.Bacc("TRN2", target_bir_lowering=False, debug=False)

    xT = nc.dram_tensor("xT", [C, T], bf16, kind="ExternalInput").ap()
    wqT = nc.dram_tensor("wqT", [C, O_G], bf16, kind="ExternalInput").ap()
    wkT = nc.dram_tensor("wkT", [C, HD], bf16, kind="ExternalInput").ap()
    wvT = nc.dram_tensor("wvT", [C, HD], bf16, kind="ExternalInput").ap()
    wpT = nc.dram_tensor("wpT", [O_G, C], bf16, kind="ExternalInput").ap()
    bq = nc.dram_tensor("bq", [HD, N_REP], f32, kind="ExternalInput").ap()
    bqs = nc.dram_tensor("bqs", [HD, N_REP], f32, kind="ExternalInput").ap()
    bk = nc.dram_tensor("bk", [HD, 1], f32, kind="ExternalInput").ap()
    bks = nc.dram_tensor("bks", [HD, 1], f32, kind="ExternalInput").ap()
    bv = nc.dram_tensor("bv", [HD, 1], f32, kind="ExternalInput").ap()
    ropeA = nc.dram_tensor("ropeA", [HD, T], f32, kind="ExternalInput").ap()
    ropeB = nc.dram_tensor("ropeB", [HD, T], f32, kind="ExternalInput").ap()
    masks = nc.dram_tensor("masks", [128, 4, TC], bf16, kind="ExternalInput").ap()
    out = nc.dram_tensor("out", [T, C], f32, kind="ExternalOutput").ap()

    Exp = mybir.ActivationFunctionType.Exp
    Copy = mybir.ActivationFunctionType.Copy

    with tile.TileContext(nc) as tc:
        import contextlib

        with contextlib.ExitStack() as ctx:
            persist = ctx.enter_context(tc.tile_pool(name="persist", bufs=1))

            # ---- persistent SBUF tensors ----
            wpT_sb = persist.tile([128, N_REP, C], bf16)
            qT_sb = persist.tile([128, N_REP, T], bf16)
            kT_sb = persist.tile([128, T], bf16)
            v_sb = persist.tile([128, N_KT, HD], bf16)
            yT_sb = persist.tile([128, N_REP, T], bf16)
            masks_sb = persist.tile([128, 4, TC], bf16)
            onescol = persist.tile([128, 1], bf16)
            onesrow = persist.tile([1, 128], bf16)
            ident = persist.tile([128, 128], bf16)

            # ---- phase B (projections): scoped pools ----
            bctx = contextlib.ExitStack()
            bpool = bctx.enter_context(tc.tile_pool(name="phase_b", bufs=1))
            tmp_pool = bctx.enter_context(tc.tile_pool(name="rope_tmp", bufs=2))
            xT_sb = bpool.tile([128, N_KT, T], bf16)
            wqT_sb = bpool.tile([128, N_KT, O_G], bf16)
            wkT_sb = bpool.tile([128, N_KT, HD], bf16)
            wvT_sb = bpool.tile([128, N_KT, HD], bf16)
            ropeA_sb = bpool.tile([128, T], f32)
            ropeB_sb = bpool.tile([128, T], f32)
            bq_sb = bpool.tile([HD, N_REP], f32)
            bqs_sb = bpool.tile([HD, N_REP], f32)
            bk_sb = bpool.tile([HD, 1], f32)
            bks_sb = bpool.tile([HD, 1], f32)
            bv_sb = bpool.tile([HD, 1], f32)
            vT_sb = bpool.tile([128, T], bf16)

            nc.vector.memset(onescol[:], 1.0)
            nc.vector.memset(onesrow[:], 1.0)
            make_identity(nc, ident[:])

            # dummy matmuls while the first DMAs land: keeps the PE busy so the
            # HAM clock-gate is already at 8/8 when real work starts
            with tc.tile_pool(name="warmpsum", bufs=1, space="PSUM") as warmpool:
                wps = warmpool.tile([128, 128], f32)
                for _ in range(48):
                    nc.tensor.matmul(wps[:], lhsT=ident[:], rhs=ident[:],
                                     start=True, stop=True)

            # per-contraction-tile loads, in consumption order, so the first
            # projection matmuls start after ~640KB instead of ~13MB
            xT_r = xT.rearrange("(kt p) t -> p kt t", p=128)
            wqT_r = wqT.rearrange("(kt p) o -> p kt o", p=128)
            for kt in range(N_KT):
                nc.sync.dma_start(wqT_sb[:, kt, :], wqT_r[:, kt, :])
                nc.sync.dma_start(xT_sb[:, kt, :], xT_r[:, kt, :])
                if kt == 0:
                    nc.sync.dma_start(bq_sb[:], bq[:])
                    nc.sync.dma_start(bqs_sb[:], bqs[:])
                    nc.sync.dma_start(bks_sb[:], bks[:])
                if kt == 3:
                    nc.sync.dma_start(ropeA_sb[:], ropeA[:])
                    nc.sync.dma_start(ropeB_sb[:], ropeB[:])
            nc.sync.dma_start(bk_sb[:], bk[:])
            nc.sync.dma_start(bv_sb[:], bv[:])
            nc.sync.dma_start(
                wkT_sb[:], wkT.rearrange("(kt p) o -> p kt o", p=128)
            )
            nc.sync.dma_start(
                wvT_sb[:], wvT.rearrange("(kt p) o -> p kt o", p=128)
            )
            nc.sync.dma_start(masks_sb[:], masks[:])
            nc.sync.dma_start(wpT_sb[:], wpT.rearrange("(h p) m -> p h m", p=128))

            def rope_epilogue(ps, dst, t0, bias, bias_sw):
                """dst (bf16 SBUF) = rope(ps + bias) using A/B tables; ps is
                fp32 psum [128, TC] at positions t0:t0+TC, bias a [128,1]
                per-partition column."""
                A = ropeA_sb[:, t0 : t0 + TC]
                Bm = ropeB_sb[:, t0 : t0 + TC]
                mult = mybir.AluOpType.mult
                add = mybir.AluOpType.add
                tmp = tmp_pool.tile([128, TC], f32, tag="rope_tmp")
                tmp2 = tmp_pool.tile([128, TC], f32, tag="rope_tmp2")
                nc.vector.scalar_tensor_tensor(
                    tmp[0:64, :], ps[64:128, :], bias_sw[0:64, :], Bm[0:64, :],
                    add, mult,
                )
                nc.vector.scalar_tensor_tensor(
                    tmp[64:128, :], ps[0:64, :], bias_sw[64:128, :], Bm[64:128, :],
                    add, mult,
                )
                nc.vector.scalar_tensor_tensor(
                    tmp2[:], ps[:], bias[:], A, add, mult
                )
                nc.vector.tensor_add(dst, tmp2[:], tmp[:])

            # ---- phase B: q/k/v projections (one shared psum pool) ----
            Identity = mybir.ActivationFunctionType.Identity
            with (
                tc.tile_pool(name="bpsum", bufs=6, space="PSUM") as bpsum,
                tc.tile_pool(name="tpsum", bufs=2, space="PSUM") as tpsum,
            ):
                for tci in range(N_TC):
                    t0 = tci * TC
                    for ot in range(N_REP):
                        ps = bpsum.tile([128, TC], f32, tag="b", name="ps_q")
                        for kt in range(N_KT):
                            nc.tensor.matmul(
                                ps[:],
                                lhsT=wqT_sb[:, kt, ot * 128 : (ot + 1) * 128],
                                rhs=xT_sb[:, kt, t0 : t0 + TC],
                                start=(kt == 0),
                                stop=(kt == N_KT - 1),
                            )
                        rope_epilogue(
                            ps, qT_sb[:, ot, t0 : t0 + TC], t0,
                            bq_sb[:, ot : ot + 1], bqs_sb[:, ot : ot + 1],
                        )
                for tci in range(N_TC):
                    t0 = tci * TC
                    psk = bpsum.tile([128, TC], f32, tag="b", name="ps_k")
                    for kt in range(N_KT):
                        nc.tensor.matmul(
                            psk[:],
                            lhsT=wkT_sb[:, kt, :],
                            rhs=xT_sb[:, kt, t0 : t0 + TC],
                            start=(kt == 0),
                            stop=(kt == N_KT - 1),
                        )
                    rope_epilogue(
                        psk, kT_sb[:, t0 : t0 + TC], t0, bk_sb[:], bks_sb[:]
                    )

                    psv = bpsum.tile([128, TC], f32, tag="b", name="ps_v")
                    for kt in range(N_KT):
                        nc.tensor.matmul(
                            psv[:],
                            lhsT=wvT_sb[:, kt, :],
                            rhs=xT_sb[:, kt, t0 : t0 + TC],
                            start=(kt == 0),
                            stop=(kt == N_KT - 1),
                        )
                    # v bias folded into the psum->sbuf cast
                    nc.scalar.activation(
                        vT_sb[:, t0 : t0 + TC], psv[:], Identity, bias=bv_sb[:]
                    )

                    # transpose this chunk's vT -> v (natural [t, d]) via PE
                    for jt in range(4 * tci, 4 * tci + 4):
                        pst = tpsum.tile([128, 128], bf16)
                        nc.tensor.transpose(
                            pst[:], vT_sb[:, jt * 128 : (jt + 1) * 128], ident[:]
                        )
                        nc.scalar.copy(v_sb[:, jt, :], pst[:])

            bctx.close()

            # ---- phase C (attention) ----
            small = ctx.enter_context(tc.tile_pool(name="small", bufs=2))
            stage_pool = ctx.enter_context(tc.tile_pool(name="stage", bufs=4))
            epool = ctx.enter_context(tc.tile_pool(name="e", bufs=3))
            cpool = ctx.enter_context(tc.tile_pool(name="cphase", bufs=1))
            dci_pool = ctx.enter_context(tc.tile_pool(name="dci", bufs=2))
            Ln = mybir.ActivationFunctionType.Ln

            # unnormalized attention outputs, fp32
            yun_sb = cpool.tile([128, N_REP, T], f32)

            with (
                tc.tile_pool(name="spsum", bufs=2, space="PSUM") as spsum,
                tc.tile_pool(name="ypsum", bufs=3, space="PSUM") as ypsum,
                tc.tile_pool(name="dpsum", bufs=1, space="PSUM") as dpsum,
            ):

                def s_group(h, i0, g):
                    """Two score matmuls (key tiles 2g, 2g+1) into one 2-bank
                    psum tile, so exp can run 1024 wide."""
                    ps = spsum.tile([128, 2, TC], f32, tag="s")
                    for sub in range(2):
                        jt = 2 * g + sub
                        nc.tensor.matmul(
                            ps[:, sub, :],
                            lhsT=kT_sb[:, jt * 128 : (jt + 1) * 128],
                            rhs=qT_sb[:, h, i0 : i0 + TC],
                            start=True,
                            stop=True,
                        )
                    return ps

                def emit_D(ci):
                    """Output projection for t-tiles of chunk ci (requires the
                    chunk's normalized yT).  PSUM slots shared with ypsum."""
                    for t_idx in range(ci * 4, ci * 4 + 4):
                        o_sb = stage_pool.tile([128, N_TC, TC], f32, tag="o_stage")
                        for mc in range(N_TC):
                            ps_o = ypsum.tile([128, TC], f32, tag="y")
                            for h in range(N_REP):
                                nc.tensor.matmul(
                                    ps_o[:],
                                    lhsT=yT_sb[
                                        :, h, t_idx * 128 : (t_idx + 1) * 128
                                    ],
                                    rhs=wpT_sb[:, h, mc * TC : (mc + 1) * TC],
                                    start=(h == 0),
                                    stop=(h == N_REP - 1),
                                )
                            if mc % 2 == 0:
                                nc.scalar.copy(o_sb[:, mc, :], ps_o[:])
                            else:
                                nc.vector.tensor_copy(o_sb[:, mc, :], ps_o[:])
                        nc.sync.dma_start(
                            out[t_idx * 128 : (t_idx + 1) * 128, :], o_sb[:]
                        )

                def make_norm_thunks(ci, den_ci, i0):
                    rden_bf = dci_pool.tile([1, N_REP * TC], bf16, tag="rden")
                    lg = dci_pool.tile([1, N_REP * TC], f32, tag="lg")

                    def lnexp():
                        nc.scalar.activation(lg[:], den_ci[:], Ln)
                        nc.scalar.activation(rden_bf[:], lg[:], Exp, scale=-1.0)

                    def mk_h(h):
                        def norm_h():
                            ps_rb = spsum.tile([128, 2, TC], f32, tag="s")
                            nc.tensor.matmul(
                                ps_rb[:, 0, :],
                                lhsT=onesrow[:],
                                rhs=rden_bf[0:1, h * TC : (h + 1) * TC],
                                start=True,
                                stop=True,
                            )
                            rb_sb = stage_pool.tile([128, TC], f32, tag="rb_stage")
                            nc.vector.tensor_copy(rb_sb[:], ps_rb[:, 0, :])
                            nc.vector.tensor_mul(
                                yT_sb[:, h, i0 : i0 + TC],
                                yun_sb[:, h, i0 : i0 + TC],
                                rb_sb[:],
                            )

                        return norm_h

                    return [lnexp] + [mk_h(h) for h in range(N_REP)]

                prev_norms = []
                # flat stream of (ci, head, group) blocks with a 2-deep
                # score-matmul lookahead that crosses head AND chunk
                # boundaries, so the PE never drains between heads
                all_blocks = []
                for ci in range(N_TC):
                    for h in range(N_REP):
                        for g in range(2 * (ci + 1)):
                            all_blocks.append((ci, h, g))
                nblk = len(all_blocks)
                s_tiles = {}

                def emit_s(b):
                    if b >= nblk:
                        return
                    ci, h, g = all_blocks[b]
                    s_tiles[b] = s_group(h, ci * TC, g)

                emit_s(0)
                emit_s(1)
                state = {}
                den_cis = {}
                for b, (ci, h, g) in enumerate(all_blocks):
                    i0 = ci * TC
                    ng = 2 * (ci + 1)
                    if g == 0 and h == 0:
                        den_cis[ci] = dci_pool.tile(
                            [1, N_REP * TC], f32, tag="den_ci", name=f"den_ci{ci}"
                        )
                    if g == 0:
                        state["ps_y"] = ypsum.tile(
                            [128, TC], f32, tag="y", name="ps_y"
                        )
                        state["ps_den"] = dpsum.tile(
                            [1, TC], f32, tag="d", name="ps_den"
                        )
                    ps_s = s_tiles.pop(b)
                    e = epool.tile([128, 2, TC], bf16)
                    nc.scalar.activation(e[:], ps_s[:], Exp, scale=SCALE)
                    dg = g - 2 * ci
                    if dg >= 0:
                        nc.vector.tensor_mul(
                            e[:], e[:], masks_sb[:, 2 * dg : 2 * dg + 2, :]
                        )
                    for sub in range(2):
                        jt = 2 * g + sub
                        nc.tensor.matmul(
                            state["ps_y"][:],
                            lhsT=v_sb[:, jt, :],
                            rhs=e[:, sub, :],
                            start=(jt == 0),
                            stop=(jt == 2 * ng - 1),
                        )
                        nc.tensor.matmul(
                            state["ps_den"][:],
                            lhsT=onescol[:],
                            rhs=e[:, sub, :],
                            start=(jt == 0),
                            stop=(jt == 2 * ng - 1),
                        )
                    emit_s(b + 2)
                    if g == ng - 1:
                        # head complete: stage unnormalized output + denominator
                        nc.vector.tensor_copy(
                            den_cis[ci][0:1, h * TC : (h + 1) * TC],
                            state["ps_den"][:],
                        )
                        nc.vector.tensor_copy(
                            yun_sb[:, h, i0 : i0 + TC], state["ps_y"][:]
                        )
                        if prev_norms:
                            prev_norms.pop(0)()
                        if h == N_REP - 1:
                            # chunk complete: finish prior chunk's norms, then
                            # its output projection, then queue this chunk's
                            while prev_norms:
                                prev_norms.pop(0)()
                            if ci > 0:
                                emit_D(ci - 1)
                            prev_norms = make_norm_thunks(ci, den_cis[ci], i0)
                while prev_norms:
                    prev_norms.pop(0)()
                emit_D(N_TC - 1)

    nc.compile()
    return nc


def _get_compiled():
    global _compiled
    if _compiled is None:
        _compiled = _build()
    return _compiled


def kernel(x, cos, sin, wq, bq, wk, bk, wv, bv, wproj):
    from concourse.bass_utils import run_bass_kernel_spmd

    nc = _get_compiled()

    x = np.asarray(x, np.float32)
    wq = np.asarray(wq, np.float32)
    bq = np.asarray(bq, np.float32)
    wk = np.asarray(wk, np.float32)
    bk = np.asarray(bk, np.float32)
    wv = np.asarray(wv, np.float32)
    bv = np.asarray(bv, np.float32)
    wproj = np.asarray(wproj, np.float32)
    cosT = np.asarray(cos, np.float32)[0, :, 0, :].T  # (64, T)
    sinT = np.asarray(sin, np.float32)[0, :, 0, :].T
    ropeA = np.ascontiguousarray(np.concatenate([cosT, cosT], 0))  # (128, T)
    ropeB = np.ascontiguousarray(np.concatenate([-sinT, sinT], 0))

    jj = np.arange(128, dtype=np.int64)[:, None, None]
    rr = np.arange(4, dtype=np.int64)[None, :, None]
    ii = np.arange(TC, dtype=np.int64)[None, None, :]
    masks = ((128 * rr + jj) <= ii).astype(BF16)  # (128, 4, 512)

    xT = [np.ascontiguousarray(x[b].T).astype(BF16) for b in range(2)]

    bq_t = [
        np.ascontiguousarray(
            bq[g * O_G : (g + 1) * O_G].reshape(N_REP, HD).T
        ).astype(np.float32)
        for g in range(4)
    ]
    bk_t = [
        bk[g * HD : (g + 1) * HD, None].astype(np.float32) for g in range(4)
    ]
    in_maps = []
    for c in range(8):
        b, g = divmod(c, 4)
        in_maps.append(
            {
                "xT": xT[b],
                "wqT": np.ascontiguousarray(
                    wq[g * O_G : (g + 1) * O_G].T
                ).astype(BF16),
                "wkT": np.ascontiguousarray(
                    wk[g * HD : (g + 1) * HD].T
                ).astype(BF16),
                "wvT": np.ascontiguousarray(
                    wv[g * HD : (g + 1) * HD].T
                ).astype(BF16),
                "wpT": np.ascontiguousarray(
                    wproj[:, g * O_G : (g + 1) * O_G].T
                ).astype(BF16),
                "bq": bq_t[g],
                "bqs": np.ascontiguousarray(
                    np.concatenate([bq_t[g][64:], bq_t[g][:64]], 0)
                ),
                "bk": bk_t[g],
                "bks": np.ascontiguousarray(
                    np.concatenate([bk_t[g][64:], bk_t[g][:64]], 0)
                ),
                "bv": bv[g * HD : (g + 1) * HD, None].astype(np.float32),
                "ropeA": ropeA,
                "ropeB": ropeB,
                "masks": masks,
            }
        )

    res = run_bass_kernel_spmd(nc, in_maps, core_ids=list(range(8)))
    parts = [res.results[c]["out"] for c in range(8)]
    out = np.stack(
        [
            parts[0] + parts[1] + parts[2] + parts[3],
            parts[4] + parts[5] + parts[6] + parts[7],
        ]
    ).astype(np.float32)
    return out



# revision 12
# speedup vs baseline: 1.0505x; 1.0505x over previous
"""Causal self-attention (GQA, rope) on 8 Trainium2 NeuronCores.

Sharding: tensor-parallel over the 4 kv-head groups x data-parallel over the
batch of 2.  Core c handles batch b = c // 4 and kv-group g = c % 4:

  - q/k/v projections for the group's 4 q-heads + 1 kv-head,
  - rope, causal flash-style attention (unnormalized softmax: e = exp(s),
    y = (e @ v) * (1 / (e @ 1)) -- safe here because scores are O(1)),
  - partial output projection out_partial = y_g @ wproj[:, cols_g].T.

The host sums the 4 group partials per batch element.

All matmuls run in bf16 with fp32 PSUM accumulation.  Activations are cast
host-side; x / weights are pre-transposed host-side so the contraction dim
lands on SBUF partitions without any on-device fp32 transposes.
"""

import numpy as np
import ml_dtypes

BF16 = ml_dtypes.bfloat16

T = 2048
C = 2048
HD = 128
N_KV = 4
N_REP = 4
O_G = N_REP * HD  # 512 q-dims per group
TC = 512  # t-chunk (psum bank width in fp32)
N_TC = T // TC  # 4
N_KT = C // 128  # 16 contraction tiles
SCALE = float(1.0 / np.sqrt(HD))

_compiled = None


def _build():
    import concourse.bacc as bacc
    import concourse.mybir as mybir
    import concourse.tile as tile
    from concourse.masks import make_identity

    f32 = mybir.dt.float32
    bf16 = mybir.dt.bfloat16

    nc = bacc.Bacc("TRN2", target_bir_lowering=False, debug=False)

    xT = nc.dram_tensor("xT", [C, T], bf16, kind="ExternalInput").ap()
    wqT = nc.dram_tensor("wqT", [C, O_G], bf16, kind="ExternalInput").ap()
    wkT = nc.dram_tensor("wkT", [C, HD], bf16, kind="ExternalInput").ap()
    wvT = nc.dram_tensor("wvT", [C, HD], bf16, kind="ExternalInput").ap()
    wpT = nc.dram_tensor("wpT", [O_G, C], bf16, kind="ExternalInput").ap()
    bq = nc.dram_tensor("bq", [HD, N_REP], f32, kind="ExternalInput").ap()
    bqs = nc.dram_tensor("bqs", [HD, N_REP], f32, kind="ExternalInput").ap()
    bk = nc.dram_tensor("bk", [HD, 1], f32, kind="ExternalInput").ap()
    bks = nc.dram_tensor("bks", [HD, 1], f32, kind="ExternalInput").ap()
    bv = nc.dram_tensor("bv", [HD, 1], f32, kind="ExternalInput").ap()
    ropeA = nc.dram_tensor("ropeA", [HD, T], f32, kind="ExternalInput").ap()
    ropeB = nc.dram_tensor("ropeB", [HD, T], f32, kind="ExternalInput").ap()
    masks = nc.dram_tensor("masks", [128, 4, TC], bf16, kind="ExternalInput").ap()
    out = nc.dram_tensor("out", [T, C], bf16, kind="ExternalOutput").ap()

    Exp = mybir.ActivationFunctionType.Exp
    Copy = mybir.ActivationFunctionType.Copy

    with tile.TileContext(nc) as tc:
        import contextlib

        with contextlib.ExitStack() as ctx:
            persist = ctx.enter_context(tc.tile_pool(name="persist", bufs=1))

            # ---- persistent SBUF tensors ----
            wpT_sb = persist.tile([128, N_REP, C], bf16)
            qT_sb = persist.tile([128, N_REP, T], bf16)
            kT_sb = persist.tile([128, T], bf16)
            v_sb = persist.tile([128, N_KT, HD], bf16)
            yT_sb = persist.tile([128, N_REP, T], bf16)
            masks_sb = persist.tile([128, 4, TC], bf16)
            onescol = persist.tile([128, 1], bf16)
            onesrow = persist.tile([1, 128], bf16)
            ident = persist.tile([128, 128], bf16)

            # ---- phase B (projections): scoped pools ----
            bctx = contextlib.ExitStack()
            bpool = bctx.enter_context(tc.tile_pool(name="phase_b", bufs=1))
            tmp_pool = bctx.enter_context(tc.tile_pool(name="rope_tmp", bufs=2))
            xT_sb = bpool.tile([128, N_KT, T], bf16)
            wqT_sb = bpool.tile([128, N_KT, O_G], bf16)
            wkT_sb = bpool.tile([128, N_KT, HD], bf16)
            wvT_sb = bpool.tile([128, N_KT, HD], bf16)
            ropeA_sb = bpool.tile([128, T], f32)
            ropeB_sb = bpool.tile([128, T], f32)
            bq_sb = bpool.tile([HD, N_REP], f32)
            bqs_sb = bpool.tile([HD, N_REP], f32)
            bk_sb = bpool.tile([HD, 1], f32)
            bks_sb = bpool.tile([HD, 1], f32)
            bv_sb = bpool.tile([HD, 1], f32)
            vT_sb = bpool.tile([128, T], bf16)

            nc.vector.memset(onescol[:], 1.0)
            nc.vector.memset(onesrow[:], 1.0)
            make_identity(nc, ident[:])

            # dummy matmuls while the first DMAs land: keeps the PE busy so the
            # HAM clock-gate is already at 8/8 when real work starts
            with tc.tile_pool(name="warmpsum", bufs=1, space="PSUM") as warmpool:
                wps = warmpool.tile([128, 128], f32)
                for _ in range(24):
                    nc.tensor.matmul(wps[:], lhsT=ident[:], rhs=ident[:],
                                     start=True, stop=True)

            # per-contraction-tile loads, in consumption order, so the first
            # projection matmuls start after ~640KB instead of ~13MB
            xT_r = xT.rearrange("(kt p) t -> p kt t", p=128)
            wqT_r = wqT.rearrange("(kt p) o -> p kt o", p=128)
            for kt in range(N_KT):
                nc.sync.dma_start(wqT_sb[:, kt, :], wqT_r[:, kt, :])
                nc.sync.dma_start(xT_sb[:, kt, :], xT_r[:, kt, :])
                if kt == 0:
                    nc.sync.dma_start(bq_sb[:], bq[:])
                    nc.sync.dma_start(bqs_sb[:], bqs[:])
                    nc.sync.dma_start(bks_sb[:], bks[:])
                if kt == 3:
                    nc.sync.dma_start(ropeA_sb[:], ropeA[:])
                    nc.sync.dma_start(ropeB_sb[:], ropeB[:])
            nc.sync.dma_start(bk_sb[:], bk[:])
            nc.sync.dma_start(bv_sb[:], bv[:])
            nc.sync.dma_start(
                wkT_sb[:], wkT.rearrange("(kt p) o -> p kt o", p=128)
            )
            nc.sync.dma_start(
                wvT_sb[:], wvT.rearrange("(kt p) o -> p kt o", p=128)
            )
            nc.sync.dma_start(masks_sb[:], masks[:])
            nc.sync.dma_start(wpT_sb[:], wpT.rearrange("(h p) m -> p h m", p=128))

            def rope_epilogue(ps, dst, t0, bias, bias_sw):
                """dst (bf16 SBUF) = rope(ps + bias) using A/B tables; ps is
                fp32 psum [128, TC] at positions t0:t0+TC, bias a [128,1]
                per-partition column."""
                A = ropeA_sb[:, t0 : t0 + TC]
                Bm = ropeB_sb[:, t0 : t0 + TC]
                mult = mybir.AluOpType.mult
                add = mybir.AluOpType.add
                tmp = tmp_pool.tile([128, TC], f32, tag="rope_tmp")
                tmp2 = tmp_pool.tile([128, TC], f32, tag="rope_tmp2")
                nc.vector.scalar_tensor_tensor(
                    tmp[0:64, :], ps[64:128, :], bias_sw[0:64, :], Bm[0:64, :],
                    add, mult,
                )
                nc.vector.scalar_tensor_tensor(
                    tmp[64:128, :], ps[0:64, :], bias_sw[64:128, :], Bm[64:128, :],
                    add, mult,
                )
                nc.vector.scalar_tensor_tensor(
                    tmp2[:], ps[:], bias[:], A, add, mult
                )
                nc.vector.tensor_add(dst, tmp2[:], tmp[:])

            # ---- phase B: q/k/v projections ----
            # q proj runs kt-OUTER over groups of 2 t-chunks (8 psum banks
            # live), so each 640KB (x,wq) kt-tile is consumed right after its
            # DMA lands instead of the first psum tile needing all 10MB.
            Identity = mybir.ActivationFunctionType.Identity
            with tc.tile_pool(name="qpsum", bufs=8, space="PSUM") as qpsum:
                for tcg in range(2):
                    pss = []
                    for tci in (2 * tcg, 2 * tcg + 1):
                        for ot in range(N_REP):
                            pss.append(
                                (
                                    tci,
                                    ot,
                                    qpsum.tile(
                                        [128, TC], f32, tag="q",
                                        name=f"ps_q{tci}_{ot}",
                                    ),
                                )
                            )
                    for kt in range(N_KT):
                        for tci, ot, ps in pss:
                            nc.tensor.matmul(
                                ps[:],
                                lhsT=wqT_sb[:, kt, ot * 128 : (ot + 1) * 128],
                                rhs=xT_sb[:, kt, tci * TC : (tci + 1) * TC],
                                start=(kt == 0),
                                stop=(kt == N_KT - 1),
                            )
                    for tci, ot, ps in pss:
                        rope_epilogue(
                            ps, qT_sb[:, ot, tci * TC : (tci + 1) * TC],
                            tci * TC,
                            bq_sb[:, ot : ot + 1], bqs_sb[:, ot : ot + 1],
                        )
            with (
                tc.tile_pool(name="bpsum", bufs=6, space="PSUM") as bpsum,
                tc.tile_pool(name="tpsum", bufs=2, space="PSUM") as tpsum,
            ):
                for tci in range(N_TC):
                    t0 = tci * TC
                    psk = bpsum.tile([128, TC], f32, tag="b", name="ps_k")
                    for kt in range(N_KT):
                        nc.tensor.matmul(
                            psk[:],
                            lhsT=wkT_sb[:, kt, :],
                            rhs=xT_sb[:, kt, t0 : t0 + TC],
                            start=(kt == 0),
                            stop=(kt == N_KT - 1),
                        )
                    rope_epilogue(
                        psk, kT_sb[:, t0 : t0 + TC], t0, bk_sb[:], bks_sb[:]
                    )

                    psv = bpsum.tile([128, TC], f32, tag="b", name="ps_v")
                    for kt in range(N_KT):
                        nc.tensor.matmul(
                            psv[:],
                            lhsT=wvT_sb[:, kt, :],
                            rhs=xT_sb[:, kt, t0 : t0 + TC],
                            start=(kt == 0),
                            stop=(kt == N_KT - 1),
                        )
                    # v bias folded into the psum->sbuf cast
                    nc.scalar.activation(
                        vT_sb[:, t0 : t0 + TC], psv[:], Identity, bias=bv_sb[:]
                    )

                    # transpose this chunk's vT -> v (natural [t, d]) via PE
                    for jt in range(4 * tci, 4 * tci + 4):
                        pst = tpsum.tile([128, 128], bf16)
                        nc.tensor.transpose(
                            pst[:], vT_sb[:, jt * 128 : (jt + 1) * 128], ident[:]
                        )
                        nc.scalar.copy(v_sb[:, jt, :], pst[:])

            bctx.close()

            # ---- phase C (attention) ----
            small = ctx.enter_context(tc.tile_pool(name="small", bufs=2))
            stage_pool = ctx.enter_context(tc.tile_pool(name="stage", bufs=4))
            epool = ctx.enter_context(tc.tile_pool(name="e", bufs=3))
            esum_pool = ctx.enter_context(tc.tile_pool(name="esum", bufs=2))
            cpool = ctx.enter_context(tc.tile_pool(name="cphase", bufs=1))
            dci_pool = ctx.enter_context(tc.tile_pool(name="dci", bufs=2))

            # unnormalized attention outputs, fp32
            yun_sb = cpool.tile([128, N_REP, T], f32)

            with (
                tc.tile_pool(name="spsum", bufs=2, space="PSUM") as spsum,
                tc.tile_pool(name="ypsum", bufs=3, space="PSUM") as ypsum,
                tc.tile_pool(name="dpsum", bufs=1, space="PSUM") as dpsum,
            ):

                def s_group(h, i0, g):
                    """Two score matmuls (key tiles 2g, 2g+1) into one 2-bank
                    psum tile, so exp can run 1024 wide."""
                    ps = spsum.tile([128, 2, TC], f32, tag="s")
                    for sub in range(2):
                        jt = 2 * g + sub
                        nc.tensor.matmul(
                            ps[:, sub, :],
                            lhsT=kT_sb[:, jt * 128 : (jt + 1) * 128],
                            rhs=qT_sb[:, h, i0 : i0 + TC],
                            start=True,
                            stop=True,
                        )
                    return ps

                def emit_D(ci):
                    """Output projection for t-tiles of chunk ci (requires the
                    chunk's normalized yT).  PSUM slots shared with ypsum."""
                    for t_idx in range(ci * 4, ci * 4 + 4):
                        o_sb = stage_pool.tile([128, N_TC, TC], bf16, tag="o_stage")
                        for mc in range(N_TC):
                            ps_o = ypsum.tile([128, TC], f32, tag="y")
                            for h in range(N_REP):
                                nc.tensor.matmul(
                                    ps_o[:],
                                    lhsT=yT_sb[
                                        :, h, t_idx * 128 : (t_idx + 1) * 128
                                    ],
                                    rhs=wpT_sb[:, h, mc * TC : (mc + 1) * TC],
                                    start=(h == 0),
                                    stop=(h == N_REP - 1),
                                )
                            if mc % 2 == 0:
                                nc.scalar.copy(o_sb[:, mc, :], ps_o[:])
                            else:
                                nc.vector.tensor_copy(o_sb[:, mc, :], ps_o[:])
                        nc.sync.dma_start(
                            out[t_idx * 128 : (t_idx + 1) * 128, :], o_sb[:]
                        )

                def make_norm_thunks(ci, den_ci, i0):
                    """The reciprocal runs on DVE (reciprocal_approx_fast),
                    keeping the scalar queue free for the exp stream (the old
                    Ln/Exp pair also forced activation-table reloads)."""
                    rden_f = dci_pool.tile([1, N_REP * TC], f32, tag="rdenf")
                    rden_bf = dci_pool.tile([1, N_REP * TC], bf16, tag="rden")

                    def recip():
                        nc.vector.reciprocal_approx_fast(rden_f[:], den_ci[:])
                        nc.vector.tensor_copy(rden_bf[:], rden_f[:])

                    def mk_h(h):
                        def norm_h():
                            ps_rb = spsum.tile([128, 2, TC], f32, tag="s")
                            nc.tensor.matmul(
                                ps_rb[:, 0, :],
                                lhsT=onesrow[:],
                                rhs=rden_bf[0:1, h * TC : (h + 1) * TC],
                                start=True,
                                stop=True,
                            )
                            rb_sb = stage_pool.tile([128, TC], f32, tag="rb_stage")
                            nc.vector.tensor_copy(rb_sb[:], ps_rb[:, 0, :])
                            nc.vector.tensor_mul(
                                yT_sb[:, h, i0 : i0 + TC],
                                yun_sb[:, h, i0 : i0 + TC],
                                rb_sb[:],
                            )

                        return norm_h

                    return [recip] + [mk_h(h) for h in range(N_REP)]

                prev_norms = []
                # flat stream of (ci, head, group) blocks with a 2-deep
                # score-matmul lookahead that crosses head AND chunk
                # boundaries, so the PE never drains between heads
                all_blocks = []
                for ci in range(N_TC):
                    for h in range(N_REP):
                        for g in range(2 * (ci + 1)):
                            all_blocks.append((ci, h, g))
                nblk = len(all_blocks)
                s_tiles = {}

                def emit_s(b):
                    if b >= nblk:
                        return
                    ci, h, g = all_blocks[b]
                    s_tiles[b] = s_group(h, ci * TC, g)

                emit_s(0)
                emit_s(1)
                state = {}
                den_cis = {}
                for b, (ci, h, g) in enumerate(all_blocks):
                    i0 = ci * TC
                    ng = 2 * (ci + 1)
                    if g == 0 and h == 0:
                        den_cis[ci] = dci_pool.tile(
                            [1, N_REP * TC], f32, tag="den_ci", name=f"den_ci{ci}"
                        )
                    if g == 0:
                        state["ps_y"] = ypsum.tile(
                            [128, TC], f32, tag="y", name="ps_y"
                        )
                        state["esum"] = esum_pool.tile(
                            [128, 2, TC], bf16, tag="esum", name="esum"
                        )
                    ps_s = s_tiles.pop(b)
                    e = epool.tile([128, 2, TC], bf16)
                    nc.scalar.activation(e[:], ps_s[:], Exp, scale=SCALE)
                    dg = g - 2 * ci
                    if dg >= 0:
                        nc.vector.tensor_mul(
                            e[:], e[:], masks_sb[:, 2 * dg : 2 * dg + 2, :]
                        )
                    # denominator: accumulate e on DVE; the PE row-sum happens
                    # once per (ci, h) row instead of per block
                    if g == 0:
                        nc.vector.tensor_copy(state["esum"][:], e[:])
                    else:
                        nc.vector.tensor_add(
                            state["esum"][:], state["esum"][:], e[:]
                        )
                    for sub in range(2):
                        jt = 2 * g + sub
                        nc.tensor.matmul(
                            state["ps_y"][:],
                            lhsT=v_sb[:, jt, :],
                            rhs=e[:, sub, :],
                            start=(jt == 0),
                            stop=(jt == 2 * ng - 1),
                        )
                    emit_s(b + 2)
                    if g == ng - 1:
                        # head complete: row-sum the accumulated e, stage
                        # unnormalized output + denominator
                        ps_den = dpsum.tile([1, TC], f32, tag="d", name="ps_den")
                        for sub in range(2):
                            nc.tensor.matmul(
                                ps_den[:],
                                lhsT=onescol[:],
                                rhs=state["esum"][:, sub, :],
                                start=(sub == 0),
                                stop=(sub == 1),
                            )
                        nc.vector.tensor_copy(
                            den_cis[ci][0:1, h * TC : (h + 1) * TC], ps_den[:]
                        )
                        nc.vector.tensor_copy(
                            yun_sb[:, h, i0 : i0 + TC], state["ps_y"][:]
                        )
                        if prev_norms:
                            prev_norms.pop(0)()
                        if h == N_REP - 1:
                            # chunk complete: finish prior chunk's norms, then
                            # its output projection, then queue this chunk's
                            while prev_norms:
                                prev_norms.pop(0)()
                            if ci > 0:
                                emit_D(ci - 1)
                            prev_norms = make_norm_thunks(ci, den_cis[ci], i0)
                while prev_norms:
                    prev_norms.pop(0)()
                emit_D(N_TC - 1)

    nc.compile()
    return nc


def _get_compiled():
    global _compiled
    if _compiled is None:
        _compiled = _build()
    return _compiled


def kernel(x, cos, sin, wq, bq, wk, bk, wv, bv, wproj):
    from concourse.bass_utils import run_bass_kernel_spmd

    nc = _get_compiled()

    x = np.asarray(x, np.float32)
    wq = np.asarray(wq, np.float32)
    bq = np.asarray(bq, np.float32)
    wk = np.asarray(wk, np.float32)
    bk = np.asarray(bk, np.float32)
    wv = np.asarray(wv, np.float32)
    bv = np.asarray(bv, np.float32)
    wproj = np.asarray(wproj, np.float32)
    cosT = np.asarray(cos, np.float32)[0, :, 0, :].T  # (64, T)
    sinT = np.asarray(sin, np.float32)[0, :, 0, :].T
    ropeA = np.ascontiguousarray(np.concatenate([cosT, cosT], 0))  # (128, T)
    ropeB = np.ascontiguousarray(np.concatenate([-sinT, sinT], 0))

    jj = np.arange(128, dtype=np.int64)[:, None, None]
    rr = np.arange(4, dtype=np.int64)[None, :, None]
    ii = np.arange(TC, dtype=np.int64)[None, None, :]
    masks = ((128 * rr + jj) <= ii).astype(BF16)  # (128, 4, 512)

    xT = [np.ascontiguousarray(x[b].T).astype(BF16) for b in range(2)]

    bq_t = [
        np.ascontiguousarray(
            bq[g * O_G : (g + 1) * O_G].reshape(N_REP, HD).T
        ).astype(np.float32)
        for g in range(4)
    ]
    bk_t = [
        bk[g * HD : (g + 1) * HD, None].astype(np.float32) for g in range(4)
    ]
    in_maps = []
    for c in range(8):
        b, g = divmod(c, 4)
        in_maps.append(
            {
                "xT": xT[b],
                "wqT": np.ascontiguousarray(
                    wq[g * O_G : (g + 1) * O_G].T
                ).astype(BF16),
                "wkT": np.ascontiguousarray(
                    wk[g * HD : (g + 1) * HD].T
                ).astype(BF16),
                "wvT": np.ascontiguousarray(
                    wv[g * HD : (g + 1) * HD].T
                ).astype(BF16),
                "wpT": np.ascontiguousarray(
                    wproj[:, g * O_G : (g + 1) * O_G].T
                ).astype(BF16),
                "bq": bq_t[g],
                "bqs": np.ascontiguousarray(
                    np.concatenate([bq_t[g][64:], bq_t[g][:64]], 0)
                ),
                "bk": bk_t[g],
                "bks": np.ascontiguousarray(
                    np.concatenate([bk_t[g][64:], bk_t[g][:64]], 0)
                ),
                "bv": bv[g * HD : (g + 1) * HD, None].astype(np.float32),
                "ropeA": ropeA,
                "ropeB": ropeB,
                "masks": masks,
            }
        )

    res = run_bass_kernel_spmd(nc, in_maps, core_ids=list(range(8)))
    parts = [res.results[c]["out"].astype(np.float32) for c in range(8)]
    out = np.stack(
        [
            parts[0] + parts[1] + parts[2] + parts[3],
            parts[4] + parts[5] + parts[6] + parts[7],
        ]
    ).astype(np.float32)
    return out



# revision 16
# speedup vs baseline: 1.1708x; 1.1146x over previous
"""Causal self-attention (GQA, rope) on 8 Trainium2 NeuronCores.

Sharding: tensor-parallel over the 4 kv-head groups x data-parallel over the
batch of 2.  Core c handles batch b = c // 4 and kv-group g = c % 4:

  - q/k/v projections for the group's 4 q-heads + 1 kv-head,
  - rope, causal flash-style attention (unnormalized softmax: e = exp(s),
    y = (e @ v) * (1 / (e @ 1)) -- safe here because scores are O(1)),
  - partial output projection out_partial = y_g @ wproj[:, cols_g].T.

The host sums the 4 group partials per batch element.

All matmuls run in bf16 with fp32 PSUM accumulation.  Activations are cast
host-side; x / weights are pre-transposed host-side so the contraction dim
lands on SBUF partitions without any on-device fp32 transposes.
"""

import numpy as np
import ml_dtypes

BF16 = ml_dtypes.bfloat16

T = 2048
C = 2048
HD = 128
N_KV = 4
N_REP = 4
O_G = N_REP * HD  # 512 q-dims per group
TC = 512  # t-chunk (psum bank width in fp32)
N_TC = T // TC  # 4
N_KT = C // 128  # 16 contraction tiles
SCALE = float(1.0 / np.sqrt(HD))

_compiled = None


def _build():
    import concourse.bacc as bacc
    import concourse.mybir as mybir
    import concourse.tile as tile
    from concourse.masks import make_identity

    f32 = mybir.dt.float32
    bf16 = mybir.dt.bfloat16

    nc = bacc.Bacc("TRN2", target_bir_lowering=False, debug=False)

    xT = nc.dram_tensor("xT", [C, T], bf16, kind="ExternalInput").ap()
    wqT = nc.dram_tensor("wqT", [C, O_G], bf16, kind="ExternalInput").ap()
    wkT = nc.dram_tensor("wkT", [C, HD], bf16, kind="ExternalInput").ap()
    wvT = nc.dram_tensor("wvT", [C, HD], bf16, kind="ExternalInput").ap()
    wpT = nc.dram_tensor("wpT", [O_G, C], bf16, kind="ExternalInput").ap()
    bq = nc.dram_tensor("bq", [HD, N_REP], f32, kind="ExternalInput").ap()
    bqs = nc.dram_tensor("bqs", [HD, N_REP], f32, kind="ExternalInput").ap()
    bk = nc.dram_tensor("bk", [HD, 1], f32, kind="ExternalInput").ap()
    bks = nc.dram_tensor("bks", [HD, 1], f32, kind="ExternalInput").ap()
    bv = nc.dram_tensor("bv", [HD, 1], f32, kind="ExternalInput").ap()
    ropeA = nc.dram_tensor("ropeA", [HD, T], f32, kind="ExternalInput").ap()
    ropeB = nc.dram_tensor("ropeB", [HD, T], f32, kind="ExternalInput").ap()
    masks = nc.dram_tensor("masks", [128, N_REP, 128], bf16, kind="ExternalInput").ap()
    out = nc.dram_tensor("out", [T, C], bf16, kind="ExternalOutput").ap()

    Exp = mybir.ActivationFunctionType.Exp
    Copy = mybir.ActivationFunctionType.Copy

    with tile.TileContext(nc) as tc:
        import contextlib

        with contextlib.ExitStack() as ctx:
            persist = ctx.enter_context(tc.tile_pool(name="persist", bufs=1))

            # ---- persistent SBUF tensors ----
            wpT_sb = persist.tile([128, N_REP, C], bf16)
            qT_sb = persist.tile([128, N_REP, T], bf16)
            kT_sb = persist.tile([128, T], bf16)
            v_sb = persist.tile([128, N_KT, HD], bf16)
            yT_sb = persist.tile([128, N_REP, T], bf16)
            masks_sb = persist.tile([128, N_REP, 128], bf16)
            onescol = persist.tile([128, 1], bf16)
            onesrow = persist.tile([1, 128], bf16)
            ident = persist.tile([128, 128], bf16)

            # ---- phase B (projections): scoped pools ----
            bctx = contextlib.ExitStack()
            bpool = bctx.enter_context(tc.tile_pool(name="phase_b", bufs=1))
            tmp_pool = bctx.enter_context(tc.tile_pool(name="rope_tmp", bufs=2))
            xT_sb = bpool.tile([128, N_KT, T], bf16)
            wqT_sb = bpool.tile([128, N_KT, O_G], bf16)
            wkT_sb = bpool.tile([128, N_KT, HD], bf16)
            wvT_sb = bpool.tile([128, N_KT, HD], bf16)
            ropeA_sb = bpool.tile([128, T], f32)
            ropeB_sb = bpool.tile([128, T], f32)
            bq_sb = bpool.tile([HD, N_REP], f32)
            bqs_sb = bpool.tile([HD, N_REP], f32)
            bk_sb = bpool.tile([HD, 1], f32)
            bks_sb = bpool.tile([HD, 1], f32)
            bv_sb = bpool.tile([HD, 1], f32)
            vT_sb = bpool.tile([128, T], bf16)

            nc.vector.memset(onescol[:], 1.0)
            nc.vector.memset(onesrow[:], 1.0)
            make_identity(nc, ident[:])

            # dummy matmuls while the first DMAs land: keeps the PE busy so the
            # HAM clock-gate is already at 8/8 when real work starts
            with tc.tile_pool(name="warmpsum", bufs=1, space="PSUM") as warmpool:
                wps = warmpool.tile([128, 128], f32)
                for _ in range(24):
                    nc.tensor.matmul(wps[:], lhsT=ident[:], rhs=ident[:],
                                     start=True, stop=True)

            # per-contraction-tile loads, in consumption order, so the first
            # projection matmuls start after ~640KB instead of ~13MB
            xT_r = xT.rearrange("(kt p) t -> p kt t", p=128)
            wqT_r = wqT.rearrange("(kt p) o -> p kt o", p=128)
            for kt in range(N_KT):
                nc.sync.dma_start(wqT_sb[:, kt, :], wqT_r[:, kt, :])
                nc.sync.dma_start(xT_sb[:, kt, :], xT_r[:, kt, :])
                if kt == 0:
                    nc.sync.dma_start(bq_sb[:], bq[:])
                    nc.sync.dma_start(bqs_sb[:], bqs[:])
                    nc.sync.dma_start(bks_sb[:], bks[:])
                if kt == 3:
                    nc.sync.dma_start(ropeA_sb[:], ropeA[:])
                    nc.sync.dma_start(ropeB_sb[:], ropeB[:])
            nc.sync.dma_start(bk_sb[:], bk[:])
            nc.sync.dma_start(bv_sb[:], bv[:])
            nc.sync.dma_start(
                wkT_sb[:], wkT.rearrange("(kt p) o -> p kt o", p=128)
            )
            nc.sync.dma_start(
                wvT_sb[:], wvT.rearrange("(kt p) o -> p kt o", p=128)
            )
            nc.sync.dma_start(masks_sb[:], masks[:])
            nc.sync.dma_start(wpT_sb[:], wpT.rearrange("(h p) m -> p h m", p=128))

            def rope_epilogue(ps, dst, t0, bias, bias_sw):
                """dst (bf16 SBUF) = rope(ps + bias) using A/B tables; ps is
                fp32 psum [128, TC] at positions t0:t0+TC, bias a [128,1]
                per-partition column.  The swap-half terms run on GpSimd so
                the psum bank frees after ~max(gpsimd, 1 DVE op) instead of
                3 serial DVE ops -- the kt-outer q sweep ends with 8 banks
                pending rope, and the next phase's matmuls wait on them."""
                A = ropeA_sb[:, t0 : t0 + TC]
                Bm = ropeB_sb[:, t0 : t0 + TC]
                mult = mybir.AluOpType.mult
                add = mybir.AluOpType.add
                tmp = tmp_pool.tile([128, TC], f32, tag="rope_tmp")
                tmp2 = tmp_pool.tile([128, TC], f32, tag="rope_tmp2")
                nc.vector.scalar_tensor_tensor(
                    tmp[0:64, :], ps[64:128, :], bias_sw[0:64, :], Bm[0:64, :],
                    add, mult,
                )
                nc.vector.scalar_tensor_tensor(
                    tmp[64:128, :], ps[0:64, :], bias_sw[64:128, :], Bm[64:128, :],
                    add, mult,
                )
                nc.vector.scalar_tensor_tensor(
                    tmp2[:], ps[:], bias[:], A, add, mult
                )
                # final add reads/writes SBUF only -> GpSimd (frees DVE, and
                # the psum bank is released after the 3 stt reads above)
                nc.gpsimd.tensor_add(dst, tmp2[:], tmp[:])

            # ---- phase B: q/k/v projections ----
            # q proj runs kt-OUTER over groups of 2 t-chunks (8 psum banks
            # live), so each 640KB (x,wq) kt-tile is consumed right after its
            # DMA lands instead of the first psum tile needing all 10MB.
            Identity = mybir.ActivationFunctionType.Identity
            with tc.tile_pool(name="qpsum", bufs=8, space="PSUM") as qpsum:
                for tcg in range(2):
                    pss = []
                    for tci in (2 * tcg, 2 * tcg + 1):
                        for ot in range(N_REP):
                            pss.append(
                                (
                                    tci,
                                    ot,
                                    qpsum.tile(
                                        [128, TC], f32, tag="q",
                                        name=f"ps_q{tci}_{ot}",
                                    ),
                                )
                            )
                    for kt in range(N_KT):
                        for tci, ot, ps in pss:
                            nc.tensor.matmul(
                                ps[:],
                                lhsT=wqT_sb[:, kt, ot * 128 : (ot + 1) * 128],
                                rhs=xT_sb[:, kt, tci * TC : (tci + 1) * TC],
                                start=(kt == 0),
                                stop=(kt == N_KT - 1),
                            )
                    for tci, ot, ps in pss:
                        rope_epilogue(
                            ps, qT_sb[:, ot, tci * TC : (tci + 1) * TC],
                            tci * TC,
                            bq_sb[:, ot : ot + 1], bqs_sb[:, ot : ot + 1],
                        )
            with (
                tc.tile_pool(name="bpsum", bufs=6, space="PSUM") as bpsum,
                tc.tile_pool(name="tpsum", bufs=2, space="PSUM") as tpsum,
            ):
                for tci in range(N_TC):
                    t0 = tci * TC
                    psk = bpsum.tile([128, TC], f32, tag="b", name="ps_k")
                    for kt in range(N_KT):
                        nc.tensor.matmul(
                            psk[:],
                            lhsT=wkT_sb[:, kt, :],
                            rhs=xT_sb[:, kt, t0 : t0 + TC],
                            start=(kt == 0),
                            stop=(kt == N_KT - 1),
                        )
                    rope_epilogue(
                        psk, kT_sb[:, t0 : t0 + TC], t0, bk_sb[:], bks_sb[:]
                    )

                    psv = bpsum.tile([128, TC], f32, tag="b", name="ps_v")
                    for kt in range(N_KT):
                        nc.tensor.matmul(
                            psv[:],
                            lhsT=wvT_sb[:, kt, :],
                            rhs=xT_sb[:, kt, t0 : t0 + TC],
                            start=(kt == 0),
                            stop=(kt == N_KT - 1),
                        )
                    # v bias folded into the psum->sbuf cast
                    nc.scalar.activation(
                        vT_sb[:, t0 : t0 + TC], psv[:], Identity, bias=bv_sb[:]
                    )

                    # transpose this chunk's vT -> v (natural [t, d]) via PE
                    for jt in range(4 * tci, 4 * tci + 4):
                        pst = tpsum.tile([128, 128], bf16)
                        nc.tensor.transpose(
                            pst[:], vT_sb[:, jt * 128 : (jt + 1) * 128], ident[:]
                        )
                        nc.scalar.copy(v_sb[:, jt, :], pst[:])

            bctx.close()

            # ---- phase C (attention) ----
            # Packed-head stream: each matmul handles all 4 q-heads x 128
            # queries (N=512).  GQA means the 4 heads share this group's
            # kv head, so k/v lhsT tiles are head-independent and causal
            # granularity drops to 128 queries (53% of T^2 vs 62.5%).
            # Rows (ci, qs) normalize + output-project per row (1-row lag).
            stage_pool = ctx.enter_context(tc.tile_pool(name="stage", bufs=4))
            epool = ctx.enter_context(tc.tile_pool(name="e", bufs=3))
            esum_pool = ctx.enter_context(tc.tile_pool(name="esum", bufs=2))
            dci_pool = ctx.enter_context(tc.tile_pool(name="dci", bufs=2))

            with (
                tc.tile_pool(name="spsum", bufs=2, space="PSUM") as spsum,
                tc.tile_pool(name="ypsum", bufs=3, space="PSUM") as ypsum,
                tc.tile_pool(name="dpsum", bufs=1, space="PSUM") as dpsum,
            ):

                def s_group(qi, g):
                    """Score matmuls for key tiles 2g(,2g+1) against the
                    packed [4 heads x 128 queries] block qi."""
                    n_sub = min(2, (qi + 1) - 2 * g)
                    ps = spsum.tile([128, 2, TC], f32, tag="s")
                    for sub in range(n_sub):
                        jt = 2 * g + sub
                        nc.tensor.matmul(
                            ps[:, sub, :],
                            lhsT=kT_sb[:, jt * 128 : (jt + 1) * 128],
                            rhs=qT_sb[:, :, qi * 128 : (qi + 1) * 128],
                            start=True,
                            stop=True,
                        )
                    return ps

                def emit_proj(qi):
                    """Output projection for t-tile qi (its row's yT is
                    normalized one row earlier)."""
                    o_sb = stage_pool.tile([128, N_TC, TC], bf16, tag="o_stage")
                    for mc in range(N_TC):
                        ps_o = ypsum.tile([128, TC], f32, tag="y")
                        for h in range(N_REP):
                            nc.tensor.matmul(
                                ps_o[:],
                                lhsT=yT_sb[:, h, qi * 128 : (qi + 1) * 128],
                                rhs=wpT_sb[:, h, mc * TC : (mc + 1) * TC],
                                start=(h == 0),
                                stop=(h == N_REP - 1),
                            )
                        if mc % 2 == 0:
                            nc.scalar.copy(o_sb[:, mc, :], ps_o[:])
                        else:
                            nc.vector.tensor_copy(o_sb[:, mc, :], ps_o[:])
                    nc.sync.dma_start(
                        out[qi * 128 : (qi + 1) * 128, :], o_sb[:]
                    )

                # flat stream of (qi, group) with a 2-deep score-matmul
                # lookahead crossing row boundaries so the PE never drains
                all_blocks = []
                for qi in range(16):
                    for g in range((qi + 2) // 2):
                        all_blocks.append((qi, g))
                nblk = len(all_blocks)
                s_tiles = {}

                def emit_s(b):
                    if b >= nblk:
                        return
                    qi, g = all_blocks[b]
                    s_tiles[b] = s_group(qi, g)

                emit_s(0)
                emit_s(1)
                state = {}
                for b, (qi, g) in enumerate(all_blocks):
                    n_tiles = qi + 1
                    ng = (n_tiles + 1) // 2
                    n_sub = min(2, n_tiles - 2 * g)
                    if g == 0:
                        state["ps_y"] = ypsum.tile(
                            [128, TC], f32, tag="y", name="ps_y"
                        )
                        state["esum"] = esum_pool.tile(
                            [128, 2, TC], bf16, tag="esum", name="esum"
                        )
                    ps_s = s_tiles.pop(b)
                    e = epool.tile([128, 2, TC], bf16)
                    nc.scalar.activation(
                        e[:, 0:n_sub, :], ps_s[:, 0:n_sub, :], Exp, scale=SCALE
                    )
                    if g == (n_tiles - 1) // 2:
                        # group holding the diagonal key tile: triangular mask
                        ds = (n_tiles - 1) % 2
                        nc.vector.tensor_mul(
                            e[:, ds, :], e[:, ds, :], masks_sb[:]
                        )
                    # denominator: accumulate e on DVE; PE row-sums once/row
                    if g == 0:
                        nc.vector.tensor_copy(
                            state["esum"][:, 0:n_sub, :], e[:, 0:n_sub, :]
                        )
                    else:
                        nc.vector.tensor_add(
                            state["esum"][:, 0:n_sub, :],
                            state["esum"][:, 0:n_sub, :],
                            e[:, 0:n_sub, :],
                        )
                    for sub in range(n_sub):
                        jt = 2 * g + sub
                        nc.tensor.matmul(
                            state["ps_y"][:],
                            lhsT=v_sb[:, jt, :],
                            rhs=e[:, sub, :],
                            start=(jt == 0),
                            stop=(jt == n_tiles - 1),
                        )
                    emit_s(b + 2)
                    if g == ng - 1:
                        # row complete: den row-sum -> 1/den -> broadcast ->
                        # normalize this row's yT in place
                        ps_d = dpsum.tile([128, TC], f32, tag="d", name="ps_d")
                        n_den = 1 if n_tiles == 1 else 2
                        for sub in range(n_den):
                            nc.tensor.matmul(
                                ps_d[0:1, :],
                                lhsT=onescol[:],
                                rhs=state["esum"][:, sub, :],
                                start=(sub == 0),
                                stop=(sub == n_den - 1),
                            )
                        rden_f = dci_pool.tile([1, TC], f32, tag="rdenf")
                        nc.vector.reciprocal_approx_fast(
                            rden_f[:], ps_d[0:1, :]
                        )
                        rden_bf = dci_pool.tile([1, TC], bf16, tag="rden")
                        nc.vector.tensor_copy(rden_bf[:], rden_f[:])
                        # broadcast 1/den to 128 partitions via PE (same bank)
                        nc.tensor.matmul(
                            ps_d[:],
                            lhsT=onesrow[:],
                            rhs=rden_bf[:],
                            start=True,
                            stop=True,
                        )
                        rb_sb = stage_pool.tile([128, TC], f32, tag="rb_stage")
                        nc.vector.tensor_copy(rb_sb[:], ps_d[:])
                        nc.vector.tensor_mul(
                            yT_sb[:, :, qi * 128 : (qi + 1) * 128],
                            state["ps_y"][:],
                            rb_sb[:],
                        )
                        if qi > 0:
                            emit_proj(qi - 1)
                emit_proj(15)

    nc.compile()
    return nc


def _get_compiled():
    global _compiled
    if _compiled is None:
        _compiled = _build()
    return _compiled


def kernel(x, cos, sin, wq, bq, wk, bk, wv, bv, wproj):
    from concourse.bass_utils import run_bass_kernel_spmd

    nc = _get_compiled()

    x = np.asarray(x, np.float32)
    wq = np.asarray(wq, np.float32)
    bq = np.asarray(bq, np.float32)
    wk = np.asarray(wk, np.float32)
    bk = np.asarray(bk, np.float32)
    wv = np.asarray(wv, np.float32)
    bv = np.asarray(bv, np.float32)
    wproj = np.asarray(wproj, np.float32)
    cosT = np.asarray(cos, np.float32)[0, :, 0, :].T  # (64, T)
    sinT = np.asarray(sin, np.float32)[0, :, 0, :].T
    ropeA = np.ascontiguousarray(np.concatenate([cosT, cosT], 0))  # (128, T)
    ropeB = np.ascontiguousarray(np.concatenate([-sinT, sinT], 0))

    jj = np.arange(128, dtype=np.int64)[:, None, None]
    ii = np.arange(128, dtype=np.int64)[None, None, :]
    masks = np.ascontiguousarray(
        np.broadcast_to(jj <= ii, (128, N_REP, 128))
    ).astype(BF16)  # triangular tile, replicated per head

    xT = [np.ascontiguousarray(x[b].T).astype(BF16) for b in range(2)]

    bq_t = [
        np.ascontiguousarray(
            bq[g * O_G : (g + 1) * O_G].reshape(N_REP, HD).T
        ).astype(np.float32)
        for g in range(4)
    ]
    bk_t = [
        bk[g * HD : (g + 1) * HD, None].astype(np.float32) for g in range(4)
    ]
    in_maps = []
    for c in range(8):
        b, g = divmod(c, 4)
        in_maps.append(
            {
                "xT": xT[b],
                "wqT": np.ascontiguousarray(
                    wq[g * O_G : (g + 1) * O_G].T
                ).astype(BF16),
                "wkT": np.ascontiguousarray(
                    wk[g * HD : (g + 1) * HD].T
                ).astype(BF16),
                "wvT": np.ascontiguousarray(
                    wv[g * HD : (g + 1) * HD].T
                ).astype(BF16),
                "wpT": np.ascontiguousarray(
                    wproj[:, g * O_G : (g + 1) * O_G].T
                ).astype(BF16),
                "bq": bq_t[g],
                "bqs": np.ascontiguousarray(
                    np.concatenate([bq_t[g][64:], bq_t[g][:64]], 0)
                ),
                "bk": bk_t[g],
                "bks": np.ascontiguousarray(
                    np.concatenate([bk_t[g][64:], bk_t[g][:64]], 0)
                ),
                "bv": bv[g * HD : (g + 1) * HD, None].astype(np.float32),
                "ropeA": ropeA,
                "ropeB": ropeB,
                "masks": masks,
            }
        )

    res = run_bass_kernel_spmd(nc, in_maps, core_ids=list(range(8)))
    parts = [res.results[c]["out"].astype(np.float32) for c in range(8)]
    out = np.stack(
        [
            parts[0] + parts[1] + parts[2] + parts[3],
            parts[4] + parts[5] + parts[6] + parts[7],
        ]
    ).astype(np.float32)
    return out

